# revision 1
# baseline (speedup 1.0000x reference)
"""Depthwise 5x5 conv (B=16, C=128, H=W=224, fp32) on 8 TRN2 NeuronCores.

Strategy
--------
Data-parallel over batch: each of the 8 cores handles 2 images.

On-core compute uses the TensorEngine via *banded matmuls* in bf16
(fp32 PSUM accumulate; quantization rel-err ~2e-3 vs 2e-2 budget):
  out[c,i,j] = sum_{u,v} k[c,u,v] * x[c, i+u-2, j+v-2] + bias[c]

The vertical taps (u) are contracted by the PE array using a host-built
block-diagonal band matrix; the horizontal taps (v) become 5 column-shifted
matmuls accumulating into the same PSUM bank.

Partition layout (contraction dim K = 128):
  partition p = 32*b + r   : block b in 0..3 holds channel c = 4*g + b,
                             r = row offset within a 32-row input strip
  psum row  m = 28*b + mr  : output row mr (0..27) of block b's channel
lhsT[p, m] = k[4g+b, r-mr, v] when p//32 == m//28 == b and 0 <= r-mr <= 4.

One iteration = one channel group g (4 channels) x one 28-row output strip
x both images. Host pre-packs the input with padding baked in: per image a
226-col segment (2 left zero cols + 224 data cols); adjacent segments share
the 2-col boundary padding; +2 trailing zeros => nw = 2*226 + 2 = 454 rhs
cols, nmm = 450 matmul free size, and output col j of image i lands at psum
col 226*i + j. G=8 iterations per DMA block give ~7KB contiguous
per-partition runs (dynamic DGE expands descriptors at ~12ns/partition-chunk,
so chunk size sets DMA bandwidth).

Engine roles (raw Bass, hand-rolled ring semaphores; this walrus build
accepts only ONE semaphore wait per TPB instruction so waits are standalone
EventSemaphore ops):
  SP HWDGE     : streams input blocks, interleaved with the weight chunks
                 + bias (tiny first chunk + first rhs block in halves) so
                 the PE starts at ~13us instead of after the full 4.6MB
                 weight preload.
  PE           : 5 banded matmuls/iteration, 8-bank PSUM ring.
  DVE          : all PSUM->SBUF evictions (+bias, cast to bf16), ~70% duty.
                 (Splitting evictions DVE/ACT was tried and REGRESSED:
                 with a third compute engine active every engine clock
                 dropped ~18% -- chip-level power/DVFS -- see dual_ev.)
  GpSimd HWDGE : streams bf16 outputs per half-block (quarters for the
                 last block => short drain tail).
"""

from contextlib import ExitStack

import ml_dtypes
import numpy as np

import concourse.bass as bass
import concourse.mybir as mybir
from concourse.bass_utils import run_bass_kernel_spmd

F32 = mybir.dt.float32
BF16 = mybir.dt.bfloat16
NP_BF16 = ml_dtypes.bfloat16

# Problem geometry (hardcoded per spec nn_Conv_53798760350153)
B, C, H, W = 16, 128, 224, 224
KK, PAD = 5, 2
N_CORES = 8
BPC = B // N_CORES  # images per core = 2

# Tiling
CB = 4            # channels per 128-partition matmul (one per 32-row block)
RB = 32           # input rows per block (= M + 4)
M = RB - KK + 1   # output rows per strip per channel = 28
MBLK = CB * M     # psum partitions used = 112
S = W + 2         # per-image rhs segment (2 shared left-pad cols + 224 data)

G = 8    # iterations per DMA block
NBB = 3  # rhs block ring depth
NP = 8   # psum ring depth (all 8 banks; must stay even for parity split)
NOB = 3  # output block ring depth
WCHUNKS = (1, 7, 8, 8, 8)  # weight groups per preload chunk (first one tiny
                           # so the PE can start right after rhs block 0)


def build_program(c=C, h=H, w=W, bpc=BPC, max_blocks=None,
                  dual_ev=True, np_banks=NP, out_eng="gpsimd"):
    """Build the per-core Bass program. All cores run the identical program
    on their own batch shard (pure data parallel, no collectives).
    max_blocks truncates the DMA-block count (benchmarking only)."""
    NPB = np_banks
    ng = c // CB           # channel groups
    ns = h // M            # row strips per image
    nit = ng * ns          # iterations
    nw = bpc * S + 2       # rhs/psum width per iteration = 454
    now = nw - KK + 1      # evicted cols per iteration (= nmm) = 450
    nmm = now              # matmul free size (all 5 shifts stay in bounds)
    import math
    gg = math.gcd(G, nit)  # iterations per DMA block
    nblk = nit // gg       # DMA blocks
    if max_blocks is not None:
        nblk = min(nblk, max_blocks)
    wcs = [0]              # weight chunk start groups
    for wch in WCHUNKS:
        wcs.append(wcs[-1] + wch)
    nwc = len(WCHUNKS)     # weight chunks
    assert h % M == 0 and c % CB == 0 and wcs[-1] == ng and NPB % 2 == 0
    assert nmm <= 512 and nw * 4 <= 2048, "psum tile must fit one bank"
    assert gg % 4 == 0, "half/quarter-block output DMA wants gg % 4 == 0"

    nc = bass.Bass()
    x_in = nc.declare_dram_parameter("x", [nit // gg, 128, gg * nw], BF16,
                                     isOutput=False)
    w_in = nc.declare_dram_parameter("w", [128, ng * KK * MBLK], BF16,
                                     isOutput=False)
    b_in = nc.declare_dram_parameter("b", [128, ng], F32, isOutput=False)
    out_t = nc.declare_dram_parameter("out", [nit // gg, MBLK, gg * now],
                                      BF16, isOutput=True)

    with ExitStack() as ctx:
        ec = ctx.enter_context
        wt = ec(nc.sbuf_tensor("wt", [128, ng * KK * MBLK], BF16))
        bt = ec(nc.sbuf_tensor("bt", [128, ng], F32))
        rhs = [ec(nc.sbuf_tensor(f"rhs{j}", [128, gg * nw], BF16))
               for j in range(NBB)]
        ot = [ec(nc.sbuf_tensor(f"ot{j}", [MBLK, gg * now], BF16))
              for j in range(NOB)]
        ps = [ec(nc.psum_tensor(f"ps{j}", [MBLK, nw], F32))
              for j in range(NPB)]

        sem_w = ec(nc.semaphore("sem_w"))
        sem_in = [ec(nc.semaphore(f"sem_in{j}")) for j in range(NBB)]
        sem_out = [ec(nc.semaphore(f"sem_out{j}")) for j in range(NOB)]
        sem_pe = ec(nc.semaphore("sem_pe"))
        sem_ev_v = ec(nc.semaphore("sem_ev_v"))  # DVE evictions (even k)
        sem_ev_a = ec(nc.semaphore("sem_ev_a"))  # ACT evictions (odd k)
        block = ec(nc.Block())

        gcol = KK * MBLK  # wt cols per group

        @block.sync
        def _(sp):
            # input stream; bias + a tiny first weight chunk lead, the
            # remaining weight chunks slot between the first rhs blocks
            # (chunk cc lands well before the PE reaches group wcs[cc],
            # without delaying rhs prefetch). rhs block 0 goes in halves
            # so the PE can start ~1.5us earlier.
            sp.dma_start(bt[:], b_in[:]).then_inc(sem_w, 16)
            sp.dma_start(wt[:, 0:wcs[1] * gcol],
                         w_in[:, 0:wcs[1] * gcol]).then_inc(sem_w, 16)
            for kb in range(nblk):
                if kb >= NBB:
                    # rhs block slot free once MM group of its last
                    # iteration has consumed it
                    sp.wait_ge(sem_pe, (kb - NBB + 1) * gg)
                if kb == 0:
                    hw_ = gg // 2 * nw
                    sp.dma_start(rhs[0][:, 0:hw_],
                                 x_in[0][:, 0:hw_]).then_inc(sem_in[0], 16)
                    sp.dma_start(rhs[0][:, hw_:2 * hw_],
                                 x_in[0][:, hw_:2 * hw_]).then_inc(
                                     sem_in[0], 16)
                else:
                    sp.dma_start(rhs[kb % NBB][:], x_in[kb]).then_inc(
                        sem_in[kb % NBB], 16)
                if kb + 1 < nwc:
                    cc = kb + 1
                    sp.dma_start(
                        wt[:, wcs[cc] * gcol:wcs[cc + 1] * gcol],
                        w_in[:, wcs[cc] * gcol:wcs[cc + 1] * gcol],
                    ).then_inc(sem_w, 16)

        def out_dma(eng, kb, hh, nparts=2):
            # DMA one 1/nparts slice of a block's outputs once its
            # evictions are done
            it0 = gg // nparts
            ee = kb * gg + it0 * (hh + 1)  # evictions needed
            if dual_ev:
                eng.wait_ge(sem_ev_v, (ee + 1) // 2)
                eng.wait_ge(sem_ev_a, ee // 2)
            else:
                eng.wait_ge(sem_ev_v, ee)
            cl, cr = hh * it0 * now, (hh + 1) * it0 * now
            eng.dma_start(
                out_t[kb][:, cl:cr], ot[kb % NOB][:, cl:cr]
            ).then_inc(sem_out[kb % NOB], 16)

        if out_eng == "gpsimd":
            @block.gpsimd
            def _(gp):
                for kb in range(nblk):
                    np_ = 4 if kb == nblk - 1 else 2  # fine tail drain
                    for hh in range(np_):
                        out_dma(gp, kb, hh, np_)

        @block.tensor
        def _(pe):
            for k in range(nblk * gg):
                kb, ki = k // gg, k % gg
                g = k // ns
                if ki == 0 or (kb == 0 and ki == gg // 2):
                    # rhs block 0 arrives in halves (extra +16 on its slot)
                    thr = 16 * (kb // NBB + 1) + (16 if kb % NBB == 0 else 0)
                    if kb == 0:
                        thr = 16 * (ki // (gg // 2) + 1)
                    pe.wait_ge(sem_in[kb % NBB], thr)
                if k % ns == 0 and g in wcs[:-1]:
                    pe.wait_ge(sem_w, 16 * (wcs.index(g) + 2))
                if k >= NPB:
                    # psum bank k%NPB freed by eviction k-NPB (same parity)
                    if dual_ev:
                        sev = sem_ev_v if k % 2 == 0 else sem_ev_a
                        pe.wait_ge(sev, (k - NPB) // 2 + 1)
                    else:
                        pe.wait_ge(sem_ev_v, k - NPB + 1)
                p = ps[k % NPB]
                for v in range(KK):
                    off = (g * KK + v) * MBLK
                    mm = nc.tensor.matmul(
                        p[:, 0:nmm],
                        wt[:, off:off + MBLK],
                        rhs[kb % NBB][:, ki * nw + v:ki * nw + v + nmm],
                        start=(v == 0),
                        stop=(v == KK - 1),
                    )
                mm.then_inc(sem_pe, 1)

        def evict(eng, k, sev):
            kb, ki = k // gg, k % gg
            g = k // ns
            eng.wait_ge(sem_pe, k + 1)
            if ki < 2 and kb >= NOB:
                # ot block slot free once both half-DMAs of kb-NOB done
                # (each half-DMA bumps by 16 => 32 per full block)
                eng.wait_ge(sem_out[kb % NOB], 32 * (kb // NOB))
            src = ps[k % NPB][:, 0:now]
            dst = ot[kb % NOB][:, ki * now:(ki + 1) * now]
            if eng is nc.vector:
                op = nc.vector.tensor_scalar(
                    dst, src, bt[0:MBLK, g:g + 1], None, mybir.AluOpType.add)
            else:
                op = nc.scalar.activation(
                    dst, src, mybir.ActivationFunctionType.Identity,
                    bias=bt[0:MBLK, g:g + 1], scale=1.0)
            op.then_inc(sev, 1)

        @block.vector
        def _(dve):
            dve.wait_ge(sem_w, 16)
            step = 2 if dual_ev else 1
            for k in range(0, nblk * gg, step):
                evict(nc.vector, k, sem_ev_v)

        @block.scalar
        def _(act):
            act.wait_ge(sem_w, 16)
            for k in range(nblk * gg):
                kb, ki = k // gg, k % gg
                if dual_ev and k % 2 == 1:
                    evict(nc.scalar, k, sem_ev_a)
                if out_eng == "scalar" and ki == gg // 2 - 1:
                    out_dma(act, kb, 0)
                if out_eng == "scalar" and ki == gg - 1:
                    out_dma(act, kb, 1)

    return nc


def host_prep(x, kern, bias_v, c=C, h=H, w=W):
    """Host-side packing: per-DMA-block contiguous bf16 input, bf16 band
    matrices, fp32 bias columns. x is the full batch [bsz, c, h, w]."""
    ng = c // CB
    ns = h // M
    nit = ng * ns
    hp = h + 2 * PAD
    nw = BPC * S + 2
    bsz = x.shape[0]

    # per-image segment: [2 zero cols][224 data]; +2 trailing zeros per row
    x_pad = np.zeros((bsz, c, hp, S), dtype=np.float32)
    x_pad[:, :, PAD:PAD + h, PAD:PAD + w] = x

    # xr[core, it=(g,s), p=(b,r), i, q] = x_pad[core*BPC+i, 4g+b, 28s+r, q]
    ncores = bsz // BPC
    xp = x_pad.reshape(ncores, BPC, ng, CB, hp, S)
    xr = np.zeros((ncores, ng, ns, CB, RB, BPC * S + 2), dtype=np.float32)
    for s in range(ns):
        # [ncores, BPC, ng, CB, RB, S] -> [ncores, ng, CB, RB, BPC, S]
        xr[:, :, s, :, :, :BPC * S] = (
            xp[:, :, :, :, M * s:M * s + RB, :]
            .transpose(0, 2, 3, 4, 1, 5).reshape(ncores, ng, CB, RB, BPC * S))
    # group gg iterations inside the partition run:
    # [ncores, nblk, gg, 128, nw] -> [ncores, nblk, 128, gg, nw]
    import math
    gg = math.gcd(G, nit)
    xr = xr.reshape(ncores, nit // gg, gg, 128, nw).transpose(0, 1, 3, 2, 4)
    xr = np.ascontiguousarray(
        xr.reshape(ncores, nit // gg, 128, gg * nw)).astype(NP_BF16)

    wd = np.zeros((128, ng * KK * MBLK), dtype=np.float32)
    mr = np.arange(M)
    for g in range(ng):
        for v in range(KK):
            col0 = (g * KK + v) * MBLK
            for b in range(CB):
                ch = CB * g + b
                for u in range(KK):
                    wd[RB * b + mr + u, col0 + M * b + mr] = kern[ch, u, v]

    bc = np.zeros((128, ng), dtype=np.float32)
    for g in range(ng):
        for b in range(CB):
            bc[M * b:M * (b + 1), g] = bias_v[CB * g + b]

    return xr, wd.astype(NP_BF16), bc


def host_post(raw, c=C, h=H, w=W):
    """Reassemble one core's [nblk, MBLK, G*now] scratch into [bpc,c,h,w]."""
    ng, ns = c // CB, h // M
    nit = ng * ns
    now = BPC * S + 2 - KK + 1
    import math
    gg = math.gcd(G, nit)
    # [nblk, MBLK, gg, now] -> [nit, MBLK, now]
    r = raw.astype(np.float32).reshape(nit // gg, MBLK, gg, now)
    r = r.transpose(0, 2, 1, 3).reshape(ng, ns, CB, M, now)
    out = np.empty((BPC, c, h, w), dtype=np.float32)
    for i in range(BPC):
        # [ng, ns, CB, M, w] -> [CB... ] channel c = CB*g + b
        ri = r[:, :, :, :, S * i:S * i + w]
        out[i] = (ri.transpose(0, 2, 1, 3, 4)
                  .reshape(ng * CB, ns * M, w))
    return out


_NC_CACHE = None

# best measured build_program config (set from A/B experiments):
# - single-engine eviction: running DVE+ACT evictions concurrently
#   downclocked every engine ~18% (chip-level power/DVFS effect)
# - output DMA ring on the (otherwise idle) ACT engine's queue: GpSimd's
#   epilogue DRAINs are ~10x costlier and added ~6us to exec time
BUILD_CFG = {"dual_ev": False, "out_eng": "scalar"}


def kernel(**inputs):
    x = np.asarray(inputs["x"], dtype=np.float32)
    kern = np.asarray(inputs["kernel"], dtype=np.float32)
    bias_v = np.asarray(inputs["bias"], dtype=np.float32)

    xr, wd, bc = host_prep(x, kern, bias_v)

    global _NC_CACHE
    if _NC_CACHE is None:
        _NC_CACHE = build_program(**BUILD_CFG)
    nc = _NC_CACHE

    in_maps = [{"x": xr[i], "w": wd, "b": bc} for i in range(N_CORES)]
    res = run_bass_kernel_spmd(nc, in_maps, core_ids=list(range(N_CORES))).results
    return np.concatenate([host_post(r["out"]) for r in res], axis=0)

